# revision 3
# baseline (speedup 1.0000x reference)
"""Trainium2 Bass kernel for nn_ClassifierWAuto (R-GCN encoder/decoder + 2-layer
RelGraphConv + mean-pool classifier), distributed over 8 NeuronCores.

Sharding: graph-aligned node shards (64 graphs/core), incoming edges live with
their destination core. Small weights replicated. Per-layer node features are
all-gathered so every core can gather arbitrary source rows locally.

Per-core layer pipeline (per 512-node destination tile t, relation r):
  A_r^T[in,n] = sum_chunks  xg_chunk^T @ S_chunk     (PE, PSUM-chained)
    xg_chunk:  [128 edges, 128] rows gathered from the all-gathered table
               (gpsimd indirect DMA, int32 indices)
    S_chunk:   [128 edges, 512] one-hot dst-selection, built on DVE via
               is_equal(iota_row, dst_local) per chunk
  agg^T[f,n]  = loop_W^T @ x^T_tile + sum_r W_r^T @ A_r^T + b  (PE, one PSUM chain)
  x'^T        = relu(agg^T)                                    (ACT)
The self-loop term uses the feature-major copy of the local shard directly
(no gather, no S). Layer-2 output is transposed on PE and fed straight into
the mean-pool matmul; classifier + softmax run on-device on [64,10].
"""

import numpy as np

P = 128
W = 512          # destination-tile width (= max PSUM free dim in fp32)
NCORES = 8

_CACHE = {}


def _shard(graph_ids, num_graphs, N):
    gpc = num_graphs // NCORES
    starts = np.searchsorted(graph_ids, np.arange(0, num_graphs + 1, gpc))
    s = starts[:-1].astype(np.int64)
    e = starts[1:].astype(np.int64)
    return s, e, gpc


def _prep(inputs):
    h = np.asarray(inputs["h"])
    src = np.asarray(inputs["src"]).astype(np.int64)
    dst = np.asarray(inputs["dst"]).astype(np.int64)
    rel = np.asarray(inputs["rel_types"]).astype(np.int64)
    gid = np.asarray(inputs["graph_ids"]).astype(np.int64)
    G = int(inputs["num_graphs"])
    N, IN = h.shape
    R = np.asarray(inputs["W1"]).shape[0]

    s, e, gpc = _shard(gid, G, N)
    ncs = (e - s).astype(np.int64)
    NS = int(-(-ncs.max() // W) * W)          # node slots per core, 512-aligned
    NT = NS // W

    pos = np.empty(N, np.int64)               # global node -> table row
    for c in range(NCORES):
        pos[s[c]:e[c]] = c * NS + np.arange(ncs[c])

    owner = np.searchsorted(e, dst, side="right")   # dst node -> owning core

    # per-core, per-(tile, rel) chunk counts
    cnt = np.zeros((NCORES, NT, R), np.int64)
    per_core = []
    for c in range(NCORES):
        m = owner == c
        es, ed, er = src[m], dst[m], rel[m]
        dl = ed - s[c]
        t = dl // W
        order = np.lexsort((dl, er, t))
        es, dl, t, er = es[order], dl[order], t[order], er[order]
        np.add.at(cnt[c], (t, er), 1)
        per_core.append((es, dl, t, er))

    chunks = -(-cnt.max(axis=0) // P)         # [NT, R] baked chunk counts
    # ensure at least.. zero is fine (group skipped entirely)
    NCH = int(chunks.sum())

    idx_all = np.zeros((NCORES, P, NCH), np.int32)
    dlc_all = np.full((NCORES, P, NCH), -1.0, np.float32)
    for c in range(NCORES):
        es, dl, t, er = per_core[c]
        ci = 0
        p0 = 0
        # edges are sorted by (t, r, dl); walk groups in the same order
        for tt in range(NT):
            for rr in range(R):
                n = cnt[c, tt, rr]
                grp_s, grp_e = p0, p0 + n
                p0 = grp_e
                nch = chunks[tt, rr]
                if nch == 0:
                    continue
                gi = pos[es[grp_s:grp_e]].astype(np.int32)
                gd = (dl[grp_s:grp_e] - tt * W).astype(np.float32)
                buf_i = np.zeros(nch * P, np.int32)
                buf_d = np.full(nch * P, -1.0, np.float32)
                buf_i[:n] = gi
                buf_d[:n] = gd
                idx_all[c, :, ci:ci + nch] = buf_i.reshape(nch, P).T
                dlc_all[c, :, ci:ci + nch] = buf_d.reshape(nch, P).T
                ci += nch
        assert ci == NCH

    # pooling matrix [NS, gpc] with 1/count, and node mask [NS]
    pool_all = np.zeros((NCORES, NS, gpc), np.float32)
    mask_all = np.zeros((NCORES, 1, NS), np.float32)
    hT_all = np.zeros((NCORES, IN, NS), np.float32)
    for c in range(NCORES):
        n = int(ncs[c])
        mask_all[c, 0, :n] = 1.0
        hT_all[c, :, :n] = h[s[c]:e[c]].T
        gl = gid[s[c]:e[c]] - c * gpc
        cnts = np.bincount(gl, minlength=gpc).astype(np.float32)
        inv = 1.0 / np.maximum(cnts, 1.0)
        pool_all[c, np.arange(n), gl] = inv[gl]

    meta = dict(N=N, IN=IN, R=R, G=G, gpc=gpc, NS=NS, NT=NT, NCH=NCH,
                chunks=tuple(map(tuple, chunks)))
    host = dict(s=s, e=e, ncs=ncs)
    arrays = dict(idx=idx_all, dlc=dlc_all, pool=pool_all, mask=mask_all, hT=hT_all)
    return meta, host, arrays


def _build(meta):
    import concourse.bacc as bacc
    import concourse.mybir as mybir
    import concourse.tile as tile
    import concourse.bass as bass

    IN, R = meta["IN"], meta["R"]
    NS, NT, NCH = meta["NS"], meta["NT"], meta["NCH"]
    gpc = meta["gpc"]
    chunks = meta["chunks"]
    INH = IN // P                                # input-dim halves (2 for 256)
    NT2 = NS // P
    C = meta["C"]
    f32 = mybir.dt.float32

    nc = bacc.Bacc("TRN2", target_bir_lowering=False, debug=False,
                   num_devices=NCORES)

    def din(name, shape, dt=f32):
        return nc.dram_tensor(name, list(shape), dt, kind="ExternalInput").ap()

    hT = din("hT", (IN, NS))
    idx = din("idx", (P, NCH), mybir.dt.int32)
    dlc = din("dlc", (P, NCH))
    encW = din("encW", (IN, P))
    encB = din("encB", (1, P))
    decW = din("decW", (P, IN))
    decB = din("decB", (1, IN))
    Wc1 = din("Wc1", (R + 1, P, P))
    b1 = din("b1", (1, P))
    Wc2 = din("Wc2", (R + 1, P, P))
    b2 = din("b2", (1, P))
    clsW = din("clsW", (P, C))
    clsB = din("clsB", (1, C))
    maskI = din("maskI", (1, NS))
    poolI = din("poolI", (NS, gpc))
    identI = din("identI", (P, P))
    iotaI = din("iotaI", (P, W))

    decT = nc.dram_tensor("decT", [IN, NS], f32, kind="ExternalOutput").ap()
    probs = nc.dram_tensor("probs", [gpc, C], f32, kind="ExternalOutput").ap()

    x_sh = nc.dram_tensor("x_sh", [NS, P], f32).ap()
    xT_d = nc.dram_tensor("xT_d", [P, NS], f32).ap()
    x_all = nc.dram_tensor("x_all", [NCORES * NS, P], f32,
                           addr_space="Shared").ap()
    x2_sh = nc.dram_tensor("x2_sh", [NS, P], f32).ap()
    x2T_d = nc.dram_tensor("x2T_d", [P, NS], f32).ap()
    x2_all = nc.dram_tensor("x2_all", [NCORES * NS, P], f32,
                            addr_space="Shared").ap()

    RELU = mybir.ActivationFunctionType.Relu
    EXP = mybir.ActivationFunctionType.Exp

    with tile.TileContext(nc) as tc:
        with (tc.tile_pool(name="const", bufs=1) as cp,
              tc.tile_pool(name="io", bufs=4) as iop,
              tc.tile_pool(name="xg", bufs=8) as xgp,
              tc.tile_pool(name="sgen", bufs=4) as sgp,
              tc.tile_pool(name="asb", bufs=3) as asb,
              tc.tile_pool(name="outs", bufs=4) as osb,
              tc.tile_pool(name="psA", bufs=2, space="PSUM") as psA,
              tc.tile_pool(name="psB", bufs=2, space="PSUM") as psB,
              tc.tile_pool(name="psT", bufs=3, space="PSUM") as psT,
              tc.tile_pool(name="psG", bufs=1, space="PSUM") as psG):

            # ---- resident constants -------------------------------------
            encW_t = cp.tile([P, INH, P], f32)
            nc.sync.dma_start(encW_t[:], encW.rearrange("(a p) f -> p a f", p=P))
            encB_t = cp.tile([1, P], f32)
            nc.sync.dma_start(encB_t[:], encB[:])
            decW_t = cp.tile([P, IN], f32)
            nc.sync.dma_start(decW_t[:], decW[:])
            decB_t = cp.tile([1, IN], f32)
            nc.sync.dma_start(decB_t[:], decB[:])
            Wc_t = [cp.tile([P, R + 1, P], f32, name=f"Wc_t{i}", tag=f"Wc_t{i}") for i in range(2)]
            nc.sync.dma_start(Wc_t[0][:], Wc1.rearrange("r k f -> k r f"))
            nc.sync.dma_start(Wc_t[1][:], Wc2.rearrange("r k f -> k r f"))
            b_t = [cp.tile([1, P], f32, name=f"b_t{i}", tag=f"b_t{i}") for i in range(2)]
            nc.sync.dma_start(b_t[0][:], b1[:])
            nc.sync.dma_start(b_t[1][:], b2[:])
            clsW_t = cp.tile([P, C], f32)
            nc.sync.dma_start(clsW_t[:], clsW[:])
            clsB_t = cp.tile([1, C], f32)
            nc.sync.dma_start(clsB_t[:], clsB[:])
            mask_t = cp.tile([1, NS], f32)
            nc.sync.dma_start(mask_t[:], maskI[:])
            ident_t = cp.tile([P, P], f32)
            nc.sync.dma_start(ident_t[:], identI[:])
            iota_t = cp.tile([P, W], f32)
            nc.sync.dma_start(iota_t[:], iotaI[:])
            idx_t = cp.tile([P, NCH], mybir.dt.int32)
            nc.sync.dma_start(idx_t[:], idx[:])
            dlc_t = cp.tile([P, NCH], f32)
            nc.sync.dma_start(dlc_t[:], dlc[:])
            ones_g = cp.tile([1, gpc], f32)
            nc.vector.memset(ones_g[:], 1.0)

            # ---- encoder + decoder --------------------------------------
            for j in range(NT2):
                cs = slice(j * P, (j + 1) * P)
                hT_j = iop.tile([P, INH, P], f32)
                nc.sync.dma_start(hT_j[:], hT.rearrange("(a p) n -> p a n", p=P)[:, :, cs])

                xT_p = psT.tile([P, P], f32, tag="tp")
                for a in range(INH):
                    nc.tensor.matmul(xT_p[:], lhsT=encW_t[:, a, :], rhs=hT_j[:, a, :],
                                     start=(a == 0), stop=False)
                nc.tensor.matmul(xT_p[:], lhsT=encB_t[:], rhs=mask_t[:, cs],
                                 start=False, stop=True)
                xT_s = osb.tile([P, P], f32, tag="xT_s")
                nc.scalar.activation(xT_s[:], xT_p[:], RELU)
                nc.sync.dma_start(xT_d[:, cs], xT_s[:])

                x_p = psT.tile([P, P], f32, tag="tp")
                for a in range(INH):
                    nc.tensor.matmul(x_p[:], lhsT=hT_j[:, a, :], rhs=encW_t[:, a, :],
                                     start=(a == 0), stop=False)
                nc.tensor.matmul(x_p[:], lhsT=mask_t[:, cs], rhs=encB_t[:],
                                 start=False, stop=True)
                x_s = osb.tile([P, P], f32, tag="x_s")
                nc.scalar.activation(x_s[:], x_p[:], RELU)
                nc.sync.dma_start(x_sh[cs, :], x_s[:])

                for a in range(INH):
                    d_p = psT.tile([P, P], f32, tag="tp")
                    nc.tensor.matmul(d_p[:], lhsT=decW_t[:, a * P:(a + 1) * P],
                                     rhs=xT_s[:], start=True, stop=False)
                    nc.tensor.matmul(d_p[:], lhsT=decB_t[:, a * P:(a + 1) * P],
                                     rhs=mask_t[:, cs], start=False, stop=True)
                    d_s = osb.tile([P, P], f32, tag="d_s")
                    nc.vector.tensor_copy(d_s[:], d_p[:])
                    nc.sync.dma_start(decT[a * P:(a + 1) * P, cs], d_s[:])

            nc.gpsimd.collective_compute(
                "AllGather", mybir.AluOpType.bypass,
                replica_groups=[list(range(NCORES))],
                ins=[x_sh.opt()], outs=[x_all.opt()])

            # ---- relational layers --------------------------------------
            hg_p = psG.tile([gpc, P], f32)

            for lyr in range(2):
                table = x_all if lyr == 0 else x2_all
                xTd = xT_d if lyr == 0 else x2T_d
                Wct = Wc_t[lyr]
                bt = b_t[lyr]
                ci = 0
                ci = sum(sum(r) for r in chunks) * lyr  # layer-2 reuses same idx
                ci = 0
                for t in range(NT):
                    ws = slice(t * W, (t + 1) * W)
                    agg_p = psB.tile([P, W], f32, tag="agg")
                    xT_t = asb.tile([P, W], f32, tag="xTt")
                    nc.sync.dma_start(xT_t[:], xTd[:, ws])
                    nc.tensor.matmul(agg_p[:], lhsT=Wct[:, R, :], rhs=xT_t[:],
                                     start=True, stop=False)
                    for r in range(R):
                        nch = chunks[t][r]
                        if nch == 0:
                            continue
                        A_p = psA.tile([P, W], f32, tag="A_p")
                        for k in range(nch):
                            xg = xgp.tile([P, P], f32)
                            nc.gpsimd.indirect_dma_start(
                                out=xg[:], out_offset=None, in_=table[:],
                                in_offset=bass.IndirectOffsetOnAxis(
                                    ap=idx_t[:, ci:ci + 1], axis=0))
                            S = sgp.tile([P, W], f32)
                            nc.vector.tensor_tensor(
                                out=S[:], in0=iota_t[:],
                                in1=dlc_t[:, ci:ci + 1].to_broadcast([P, W]),
                                op=mybir.AluOpType.is_equal)
                            nc.tensor.matmul(A_p[:], lhsT=xg[:], rhs=S[:],
                                             start=(k == 0), stop=(k == nch - 1))
                            ci += 1
                        A_s = asb.tile([P, W], f32, tag="A_s")
                        nc.vector.tensor_copy(A_s[:], A_p[:])
                        nc.tensor.matmul(agg_p[:], lhsT=Wct[:, r, :], rhs=A_s[:],
                                         start=False, stop=False)
                    nc.tensor.matmul(agg_p[:], lhsT=bt[:], rhs=mask_t[:, ws],
                                     start=False, stop=True)
                    y_s = osb.tile([P, W], f32, tag="y_s")
                    nc.scalar.activation(y_s[:], agg_p[:], RELU)
                    if lyr == 0:
                        nc.sync.dma_start(x2T_d[:, ws], y_s[:])
                    for q in range(4):
                        tp = psT.tile([P, P], f32, tag="tp")
                        nc.tensor.transpose(tp[:], y_s[:, q * P:(q + 1) * P],
                                            ident_t[:])
                        yq = osb.tile([P, P], f32, tag="yq")
                        nc.vector.tensor_copy(yq[:], tp[:])
                        j = t * 4 + q
                        if lyr == 0:
                            nc.sync.dma_start(x2_sh[j * P:(j + 1) * P, :], yq[:])
                        else:
                            P_t = iop.tile([P, gpc], f32, tag="P_t")
                            nc.sync.dma_start(P_t[:], poolI[j * P:(j + 1) * P, :])
                            nc.tensor.matmul(hg_p[:], lhsT=P_t[:], rhs=yq[:],
                                             start=(j == 0), stop=(j == NT * 4 - 1))
                if lyr == 0:
                    nc.gpsimd.collective_compute(
                        "AllGather", mybir.AluOpType.bypass,
                        replica_groups=[list(range(NCORES))],
                        ins=[x2_sh.opt()], outs=[x2_all.opt()])

            # ---- classifier + softmax -----------------------------------
            hg_s = osb.tile([gpc, P], f32, tag="cls")
            nc.vector.tensor_copy(hg_s[:], hg_p[:])
            hgT_p = psT.tile([P, gpc], f32, tag="tp")
            nc.tensor.transpose(hgT_p[:], hg_s[:], ident_t[:gpc, :gpc])
            hgT_s = osb.tile([P, gpc], f32, tag="cls2")
            nc.vector.tensor_copy(hgT_s[:], hgT_p[:])
            lg_p = psT.tile([gpc, C], f32, tag="tp")
            nc.tensor.matmul(lg_p[:], lhsT=hgT_s[:], rhs=clsW_t[:],
                             start=True, stop=False)
            nc.tensor.matmul(lg_p[:], lhsT=ones_g[:], rhs=clsB_t[:],
                             start=False, stop=True)
            mx = osb.tile([gpc, 1], f32, tag="mx")
            nc.vector.tensor_reduce(mx[:], lg_p[:], op=mybir.AluOpType.max,
                                    axis=mybir.AxisListType.X, negate=True)
            ex = osb.tile([gpc, C], f32, tag="ex")
            nc.scalar.activation(ex[:], lg_p[:], EXP, bias=mx[:, 0:1])
            sm = osb.tile([gpc, 1], f32, tag="sm")
            nc.vector.tensor_reduce(sm[:], ex[:], op=mybir.AluOpType.add,
                                    axis=mybir.AxisListType.X)
            rs = osb.tile([gpc, 1], f32, tag="rs")
            nc.vector.reciprocal(rs[:], sm[:])
            pr = osb.tile([gpc, C], f32, tag="pr")
            nc.vector.tensor_scalar_mul(pr[:], ex[:], rs[:, 0:1])
            nc.sync.dma_start(probs[:], pr[:])

    nc.compile()
    return nc


def _run(nc, meta, inputs, arrays):
    from concourse import bass_utils
    R = meta["R"]
    w1 = np.asarray(inputs["W1"], np.float32)
    w2 = np.asarray(inputs["W2"], np.float32)
    wc1 = np.concatenate([w1, np.asarray(inputs["loop1"], np.float32)[None]], 0)
    wc2 = np.concatenate([w2, np.asarray(inputs["loop2"], np.float32)[None]], 0)
    iota = np.broadcast_to(np.arange(W, dtype=np.float32), (P, W)).copy()
    ident = np.eye(P, dtype=np.float32)
    shared = dict(
        encW=np.asarray(inputs["enc_W"], np.float32),
        encB=np.asarray(inputs["enc_b"], np.float32)[None, :],
        decW=np.asarray(inputs["dec_W"], np.float32),
        decB=np.asarray(inputs["dec_b"], np.float32)[None, :],
        Wc1=wc1, b1=np.asarray(inputs["b1"], np.float32)[None, :],
        Wc2=wc2, b2=np.asarray(inputs["b2"], np.float32)[None, :],
        clsW=np.asarray(inputs["cls_W"], np.float32),
        clsB=np.asarray(inputs["cls_b"], np.float32)[None, :],
        identI=ident, iotaI=iota,
    )
    in_maps = []
    for c in range(NCORES):
        m = dict(shared)
        m["hT"] = arrays["hT"][c]
        m["idx"] = arrays["idx"][c]
        m["dlc"] = arrays["dlc"][c]
        m["maskI"] = arrays["mask"][c]
        m["poolI"] = arrays["pool"][c]
        in_maps.append(m)
    res = bass_utils.run_bass_kernel_spmd(nc, in_maps, core_ids=list(range(NCORES)))
    return res.results


def kernel(**inputs):
    meta, host, arrays = _prep(inputs)
    meta["C"] = int(np.asarray(inputs["cls_W"]).shape[1])
    key = (meta["N"], meta["IN"], meta["R"], meta["G"], meta["NS"],
           meta["NCH"], meta["C"], meta["chunks"])
    if key not in _CACHE:
        _CACHE[key] = _build(meta)
    nc = _CACHE[key]
    results = _run(nc, meta, inputs, arrays)

    N, IN, G = meta["N"], meta["IN"], meta["G"]
    gpc = meta["gpc"]
    s, ncs = host["s"], host["ncs"]
    decoded = np.empty((N, IN), np.float32)
    probs = np.empty((G, meta["C"]), np.float32)
    for c in range(NCORES):
        n = int(ncs[c])
        decoded[s[c]:s[c] + n] = results[c]["decT"][:, :n].T
        probs[c * gpc:(c + 1) * gpc] = results[c]["probs"]
    return decoded, probs
